# revision 1
# baseline (speedup 1.0000x reference)
"""Trainium2 Bass kernel for nn_AbstractSNClustering (moe_routing).

Reference computation (B=1048576, DX=32, DS=16, H=128, K=64, NSN=4):
    x_tune   = sigmoid(hidden @ W_tune + b_tune)                  [B,1]
    dist     = ||c_k||^2 - 2 x.c_k ; cl = argmin_k dist           [B]
    x_sn     = einsum(W_sn[:,cl,:], s) + b_sn[:,cl].T             [B,NSN]
    x_sn_sum = sum(x_sn * rsw[cl], -1)                            [B,1]
    out      = x_sn_sum + x_tune * (naive_pred - x_sn_sum)

Key algebraic collapse: both the mixture weights and subnet params are
indexed by the same cluster id, so
    x_sn_sum[b] = s[b].WW[cl] + BB[cl],
    WW[k,d] = sum_n rsw[k,n] W_sn[n,k,d],  BB[k] = sum_n rsw[k,n] b_sn[n,k]
(host-precomputed).  On device, one matmul per 128-row tile against a
constant [50,128] table computes both dist[b,k] and t[b,k]=s'.WW[k]+BB[k].
Routing is DVE-only: reduce_min, one tensor_tensor is_equal with
broadcast row-min (one-hot), mult + reduce_add (select t at argmin).
Tune head z = hidden.W_tune via fused scalar_tensor_tensor accumulate.

Performance structure (measured on HW via For_i-slope timing):
  - inputs x,s (+ the two ones columns folding in ||c||^2 and BB) are
    pre-concatenated host-side into one [B,50] tensor -> one contiguous
    DMA per 4096-row macro-chunk (strided SBUF writes were 3x slow).
  - 4096-row DMA macro-chunks (5 dma_starts per macro; many small DMAs
    cost ~0.9us each), hidden on the scalar-HWDGE ring, rest on sync.
  - compute in 1024-row sub-chunks: PE transposes [128,50]->[50,128],
    per-128-row matmul vs the table, DVE routing, ACT sigmoid.
  ~648us/exec vs ~291us DMA-only floor (theoretical ~253us at 358GB/s).

Sharding: pure data parallel over B across 8 NeuronCores; parameter
tables replicated.
"""

import contextlib
import os
import sys

sys.path.insert(0, "/opt/trn_rl_repo")

import numpy as np

import concourse.bass as bass
import concourse.mybir as mybir
from concourse import bacc, tile
from concourse.bass_utils import run_bass_kernel_spmd

B, DX, DS, H, K, NSN = 1048576, 32, 16, 128, 64, 4
NCORES = 8
ROWS = B // NCORES          # rows per core
CHUNK = int(os.environ.get("K_CHUNK", "1024"))  # rows per compute sub-chunk
CPT = CHUNK // 128          # sub-rows per partition per sub-chunk
MCH = int(os.environ.get("K_MCH", "4096"))     # rows per DMA macro-chunk
NF = DX + 1 + DS + 1        # 50 contraction features: [x | 1 | s | 1]
XCOL1 = DX                  # ones col for x (c2 row)
SCOL0 = DX + 1              # s cols start
SCOL1 = DX + 1 + DS         # ones col for s (BB row)

f32 = mybir.dt.float32
bf16 = mybir.dt.bfloat16
Alu = mybir.AluOpType
Act = mybir.ActivationFunctionType
AX = mybir.AxisListType

# tuning knobs (env-overridable for A/B benching)
ISEQ_MODE = os.environ.get("K_ISEQ", "tt_bcast")   # "ts" | "tt_bcast"
Z_MODE = os.environ.get("K_Z", "stt")         # "tt" | "stt"
HID_BF16 = os.environ.get("K_HBF16", "0") == "1"
XSN_MODE = os.environ.get("K_XSN", "tt")     # "tt" | "stt"
BLEND_GPSIMD = os.environ.get("K_BLGP", "0") == "1"  # blend ops on GPSIMD
IO_BUFS = int(os.environ.get("K_IOB", "2"))
MID_BUFS = int(os.environ.get("K_MIDB", "3"))
DMASK = int(os.environ.get("K_DMASK", "15"))  # stage0 DMA attribution mask
USE_TTR = False     # fused tensor_tensor_reduce (crashes device on this runtime)


def _build(
    rows: int, b_tune_val: float, stage: int = 99, reps: int = 1
) -> bass.Bass:
    """stage: 0=DMA only, 99=full
    reps: on-device repeat count (timing only; >1 wraps the kernel in For_i)"""
    mch = min(MCH, rows)            # rows per DMA macro-chunk
    cpm = mch // 128                # rows per partition per macro
    nsub = mch // CHUNK             # compute sub-chunks per macro
    nmacro = rows // mch
    assert rows % mch == 0 and mch % CHUNK == 0
    nc = bacc.Bacc(None)

    hdt = bf16 if HID_BF16 else f32
    xs_ext = nc.declare_dram_parameter("xs1", [rows, NF], f32, isOutput=False)
    h_ext = nc.declare_dram_parameter("hidden", [rows, H], f32, isOutput=False)
    nv_ext = nc.declare_dram_parameter("naive", [rows, 1], f32, isOutput=False)
    tbl_ext = nc.declare_dram_parameter("bigtable", [NF, 128], f32, isOutput=False)
    idn_ext = nc.declare_dram_parameter("ident", [128, 128], f32, isOutput=False)
    wtr_ext = nc.declare_dram_parameter("wtune_rep", [128, H], hdt, isOutput=False)
    out_ext = nc.declare_dram_parameter("out", [rows, 1], f32, isOutput=True)

    with tile.TileContext(nc) as tc:
        with (
            tc.tile_pool(name="consts", bufs=1) as cpool,
            tc.tile_pool(name="io", bufs=IO_BUFS) as io,
            tc.tile_pool(name="mid", bufs=MID_BUFS) as mid,
            tc.tile_pool(name="pst", bufs=2, space="PSUM") as pst,
            tc.tile_pool(name="psm", bufs=2, space="PSUM") as psm,
        ):
            ident = cpool.tile([128, 128], f32, tag="ident")
            nc.sync.dma_start(ident[:], idn_ext[:])
            btbl = cpool.tile([NF, 128], f32, tag="btbl")
            nc.sync.dma_start(btbl[:], tbl_ext[:])
            wtr = cpool.tile([128, H], hdt, tag="wtr")
            nc.sync.dma_start(wtr[:], wtr_ext[:])
            btn = cpool.tile([128, 1], f32, tag="btn")
            nc.vector.memset(btn[:], b_tune_val)

            loop_cm = tc.For_i(0, reps, 1) if reps > 1 else contextlib.nullcontext()
            with loop_cm:
              for m in range(nmacro):
                m0 = m * mch
                ct = io.tile([128, cpm * NF], f32, tag="catm")
                c3m = ct[:].rearrange("p (c f) -> p c f", c=cpm)

                # macro DMAs: row b = m0 + p*cpm + c  (per-partition contiguous)
                if stage >= 1 or DMASK & 1:
                    nc.sync.dma_start(
                        ct[:],
                        xs_ext[m0 : m0 + mch, :].rearrange(
                            "(p c) f -> p (c f)", c=cpm
                        ),
                    )
                else:
                    nc.vector.memset(ct[:, 0:1], 0.0)
                htm = io.tile([128, cpm * H], hdt, tag="htm")
                h_src = h_ext[m0 : m0 + mch, :].rearrange("(p c) d -> p (c d)", c=cpm)
                if stage >= 1 or DMASK & 4:
                    if HID_BF16:
                        nc.gpsimd.dma_start(htm[:], h_src)   # SWDGE dtype-cast DMA
                    else:
                        nc.scalar.dma_start(htm[:], h_src)   # second HWDGE ring
                elif stage < 1:
                    nc.vector.memset(htm[:, 0:1], 0.0)
                h3m = htm[:].rearrange("p (c f) -> p c f", c=cpm)
                nvm = io.tile([128, cpm], f32, tag="nvm")
                nc.sync.dma_start(
                    nvm[:],
                    nv_ext[m0 : m0 + mch, :].rearrange("(p c) o -> p (c o)", c=cpm),
                )
                resm = mid.tile([128, cpm], f32, tag="resm")

                if stage < 1:
                    nc.vector.tensor_copy(resm[:], nvm[:])
                    nc.vector.tensor_add(resm[:, 0:1], resm[:, 0:1], htm[:, 0:1])
                    nc.vector.tensor_add(resm[:, 0:1], resm[:, 0:1], ct[:, 0:1])
                else:
                  for sc in range(nsub):
                    c0 = sc * CPT
                    # [x|1|s|1] -> transposed [50, 128] per 128-row sub-tile
                    xsT_ps = pst.tile([NF, CPT * 128], f32, tag="xsT_ps")
                    for c in range(CPT):
                        nc.tensor.transpose(
                            xsT_ps[:, c * 128 : (c + 1) * 128],
                            c3m[:, c0 + c, :],
                            ident[:],
                        )
                    xsT = mid.tile([NF, CPT * 128], f32, tag="xsT")
                    nc.scalar.copy(xsT[:], xsT_ps[:])

                    # one matmul per sub-tile: out cols = [dist(64) | t(64)]
                    dt_ps = psm.tile([128, CPT * 128], f32, tag="dt_ps")
                    d3 = dt_ps[:].rearrange("p (c f) -> p c f", c=CPT)
                    for c in range(CPT):
                        nc.tensor.matmul(
                            d3[:, c, :],
                            xsT[:, c * 128 : (c + 1) * 128],
                            btbl[:],
                            start=True,
                            stop=True,
                        )

                    # routing: row-min over k, one-hot, select t at argmin
                    rmin = mid.tile([128, CPT], f32, tag="rmin")
                    nc.vector.tensor_reduce(
                        rmin[:], d3[:, :, 0:K], axis=AX.X, op=Alu.min
                    )
                    xsn = mid.tile([128, CPT], f32, tag="xsn")
                    scr = mid.tile([128, CPT * K], f32, tag="scr")
                    sc3 = scr[:].rearrange("p (c f) -> p c f", c=CPT)
                    if XSN_MODE == "stt2":
                        # fused routing: ACT copies t to SBUF, then one DVE
                        # scalar_tensor_tensor per sub-tile computes
                        # (dist == rmin) * t with accumulate -> xsn
                        tsb = mid.tile([128, CPT * K], f32, tag="tsb")
                        ts3 = tsb[:].rearrange("p (c f) -> p c f", c=CPT)
                        nc.scalar.copy(ts3[:, :, :], d3[:, :, K : 2 * K])
                        for c in range(CPT):
                            nc.vector.scalar_tensor_tensor(
                                out=sc3[:, c, :],
                                in0=d3[:, c, 0:K],
                                scalar=rmin[:, c : c + 1],
                                in1=ts3[:, c, :],
                                op0=Alu.is_equal,
                                op1=Alu.mult,
                                accum_out=xsn[:, c : c + 1],
                            )
                    if XSN_MODE != "stt2":
                        oh = mid.tile([128, CPT * K], f32, tag="oh")
                        oh3 = oh[:].rearrange("p (c f) -> p c f", c=CPT)
                    if XSN_MODE == "stt2":
                        pass
                    elif ISEQ_MODE == "tt_bcast":
                        rb = rmin[:].unsqueeze(2).broadcast_to([128, CPT, K])
                        nc.vector.tensor_tensor(
                            oh3[:, :, :], d3[:, :, 0:K], rb, op=Alu.is_equal
                        )
                    else:
                        for c in range(CPT):
                            nc.vector.tensor_scalar(
                                oh3[:, c, :], d3[:, c, 0:K],
                                rmin[:, c : c + 1], None, op0=Alu.is_equal,
                            )
                    if XSN_MODE == "stt2":
                        pass
                    elif XSN_MODE == "stt":
                        for c in range(CPT):
                            nc.vector.scalar_tensor_tensor(
                                out=sc3[:, c, :],
                                in0=d3[:, c, K : 2 * K],
                                scalar=1.0,
                                in1=oh3[:, c, :],
                                op0=Alu.bypass,
                                op1=Alu.mult,
                                accum_out=xsn[:, c : c + 1],
                            )
                    else:
                        nc.vector.tensor_mul(
                            sc3[:, :, :], d3[:, :, K : 2 * K], oh3[:, :, :]
                        )
                        nc.vector.tensor_reduce(
                            xsn[:], sc3[:, :, :], axis=AX.X, op=Alu.add
                        )

                    # tune head: z = hidden @ W_tune, sigmoid(z + b_tune)
                    z = mid.tile([128, CPT], f32, tag="z")
                    scz = mid.tile([128, CPT * H], hdt, tag="scz")
                    z3 = scz[:].rearrange("p (c f) -> p c f", c=CPT)
                    if Z_MODE == "stt":
                        for c in range(CPT):
                            nc.vector.scalar_tensor_tensor(
                                out=z3[:, c, :],
                                in0=h3m[:, c0 + c, :],
                                scalar=1.0,
                                in1=wtr[:],
                                op0=Alu.bypass,
                                op1=Alu.mult,
                                accum_out=z[:, c : c + 1],
                            )
                    elif Z_MODE == "gpstt":
                        # fully fused on GPSIMD: mult + accumulate
                        for c in range(CPT):
                            nc.gpsimd.scalar_tensor_tensor(
                                out=z3[:, c, :],
                                in0=h3m[:, c0 + c, :],
                                scalar=1.0,
                                in1=wtr[:],
                                op0=Alu.bypass,
                                op1=Alu.mult,
                                accum_out=z[:, c : c + 1],
                            )
                    elif Z_MODE == "gp":
                        # multiply on GPSIMD (otherwise idle), reduce on DVE
                        wb = wtr[:].unsqueeze(1).broadcast_to([128, CPT, H])
                        nc.gpsimd.tensor_mul(
                            z3[:, :, :], h3m[:, c0 : c0 + CPT, :], wb
                        )
                        nc.vector.tensor_reduce(
                            z[:], z3[:, :, :], axis=AX.X, op=Alu.add
                        )
                    else:
                        for c in range(CPT):
                            nc.vector.tensor_mul(
                                z3[:, c, :], h3m[:, c0 + c, :], wtr[:]
                            )
                        nc.vector.tensor_reduce(
                            z[:], z3[:, :, :], axis=AX.X, op=Alu.add
                        )
                    sig = mid.tile([128, CPT], f32, tag="sig")
                    nc.scalar.activation(sig[:], z[:], Act.Sigmoid, bias=btn[:, 0:1])

                    # blend: out = xsn + sig * (naive - xsn)
                    eng = nc.gpsimd if BLEND_GPSIMD else nc.vector
                    dd = mid.tile([128, CPT], f32, tag="dd")
                    eng.tensor_sub(dd[:], nvm[:, c0 : c0 + CPT], xsn[:])
                    mm = mid.tile([128, CPT], f32, tag="mm")
                    eng.tensor_mul(mm[:], dd[:], sig[:])
                    eng.tensor_add(resm[:, c0 : c0 + CPT], xsn[:], mm[:])

                nc.scalar.dma_start(
                    out_ext[m0 : m0 + mch, :].rearrange("(p c) o -> p (c o)", c=cpm),
                    resm[:],
                )
    if not nc.is_finalized():
        nc.finalize()
    return nc


def _prep_tables(centers, W_tune, b_tune, W_sn, b_sn, running_sn_weight):
    centers = np.asarray(centers, np.float32)
    W_sn = np.asarray(W_sn, np.float32)
    b_sn = np.asarray(b_sn, np.float32)
    rsw = np.asarray(running_sn_weight, np.float32)
    c2 = (centers * centers).sum(1)                      # [K]
    WW = np.einsum("kn,nkd->kd", rsw, W_sn)              # [K, DS]
    BB = np.einsum("kn,nk->k", rsw, b_sn)                # [K]
    tbl = np.zeros((NF, 128), np.float32)
    tbl[0:DX, 0:K] = -2.0 * centers.T                    # dist linear term
    tbl[XCOL1, 0:K] = c2                                 # dist const term
    tbl[SCOL0:SCOL1, K : 2 * K] = WW.T                   # t linear term
    tbl[SCOL1, K : 2 * K] = BB                           # t const term
    ident = np.eye(128, dtype=np.float32)
    wtr = np.broadcast_to(
        np.asarray(W_tune, np.float32).reshape(1, H), (128, H)
    ).copy()
    if HID_BF16:
        import ml_dtypes

        wtr = wtr.astype(ml_dtypes.bfloat16)
    return tbl, ident, wtr, float(np.asarray(b_tune).reshape(-1)[0])


def make_xs1(x, s):
    rows = x.shape[0]
    xs1 = np.empty((rows, NF), np.float32)
    xs1[:, 0:DX] = x
    xs1[:, XCOL1] = 1.0
    xs1[:, SCOL0:SCOL1] = s
    xs1[:, SCOL1] = 1.0
    return xs1


def make_in_maps(inputs, rows_per_core, n_cores=NCORES):
    x = np.asarray(inputs["x"], np.float32)
    s = np.asarray(inputs["s"], np.float32)
    hidden = np.ascontiguousarray(np.asarray(inputs["hidden"], np.float32))
    naive = np.ascontiguousarray(np.asarray(inputs["naive_pred"], np.float32))
    xs1 = make_xs1(x, s)
    tbl, ident, wtr, b_tune_val = _prep_tables(
        inputs["centers"], inputs["W_tune"], inputs["b_tune"],
        inputs["W_sn"], inputs["b_sn"], inputs["running_sn_weight"],
    )
    in_maps = []
    for i in range(n_cores):
        r0 = i * rows_per_core
        in_maps.append(
            {
                "xs1": xs1[r0 : r0 + rows_per_core],
                "hidden": hidden[r0 : r0 + rows_per_core],
                "naive": naive[r0 : r0 + rows_per_core],
                "bigtable": tbl,
                "ident": ident,
                "wtune_rep": wtr,
            }
        )
    return in_maps, b_tune_val


def _run(inputs, rows_per_core=ROWS, n_cores=NCORES, trace=False, tmpdir=None):
    in_maps, b_tune_val = make_in_maps(inputs, rows_per_core, n_cores)
    nc = _build(rows_per_core, b_tune_val)
    bres = run_bass_kernel_spmd(
        nc, in_maps, core_ids=list(range(n_cores)), trace=trace, tmpdir=tmpdir
    )
    out = np.concatenate([r["out"] for r in bres.results], axis=0)
    return out, bres


def kernel(**inputs) -> np.ndarray:
    out, _ = _run(inputs)
    return out



# revision 2
# speedup vs baseline: 14.9885x; 14.9885x over previous
"""Trainium2 Bass kernel V2 for nn_AbstractSNClustering (moe_routing).

Reference computation (B=1048576, DX=32, DS=16, H=128, K=64, NSN=4):
    x_tune   = sigmoid(hidden @ W_tune + b_tune)                  [B,1]
    dist     = ||c_k||^2 - 2 x.c_k ; cl = argmin_k dist           [B]
    x_sn     = einsum(W_sn[:,cl,:], s) + b_sn[:,cl].T             [B,NSN]
    x_sn_sum = sum(x_sn * rsw[cl], -1)                            [B,1]
    out      = x_sn_sum + x_tune * (naive_pred - x_sn_sum)

V2 design vs the 702us baseline:
  * All inputs cast to fp16 host-side (validated: rel err 9.0e-3 < 2e-2
    gate; bf16 fails at 2.5e-2 from argmin flips).  Halves DMA bytes and
    makes every PE matmul run at 1 cycle/row.
  * Inputs are pre-TRANSPOSED host-side into [features, rows] layouts so
    the PE consumes them directly as stationary operands: no on-device
    transposes, no PSUM->SBUF staging copies of transposed data.
  * xs features (x|1|s|1 = 50) are packed two row-tiles per 128
    partitions (bases 0 and 64 - LDW base partition must be 0/32/64/96)
    and DMA'd as two 50-partition transfers (no pad bytes).
  * z = hidden @ W_tune moved from DVE to PE (hT tile stationary,
    W_tune the single moving column).
  * Routing epilogue minimized: rmin (tensor_reduce, PSUM f32) ->
    one-hot is_equal (tensor_tensor vs broadcast rmin) -> select via
    copy_predicated with stride-0 output (or per-tile fused stt) ->
    3-op blend.  Sigmoid + t-copy on ACT.
Sharding: pure data parallel over B across 8 cores; tables replicated.
"""

import contextlib
import os
import sys

sys.path.insert(0, "/opt/trn_rl_repo")

import numpy as np

import concourse.bass as bass
import concourse.mybir as mybir
from concourse import bacc, tile
from concourse.bass_utils import run_bass_kernel_spmd

B, DX, DS, H, K, NSN = 1048576, 32, 16, 128, 64, 4
NCORES = 8
ROWS = B // NCORES
NF = DX + 1 + DS + 1        # 50 features: [x | 1 | s | 1]
CHUNK = int(os.environ.get("K_CHUNK", "1024"))   # rows per compute sub-chunk
CPT = CHUNK // 128
MCH = int(os.environ.get("K_MCH", "4096"))       # rows per DMA macro-chunk
H8 = os.environ.get("K_H8", "0") == "1"          # hidden in fp8 (e4m3)
SEL_MODE = os.environ.get("K_SEL", "stt2")       # stt2 | cpred (cpred: stride-0 out writes do not land; broken)
NOZ = os.environ.get("K_NOZ", "0") == "1"        # debug: skip z matmul
EVENONLY = os.environ.get("K_EVENONLY", "0") == "1"  # debug: even tiles only
TILES_PER_PART = 2                               # xs tiles packed per 128 partitions

f32 = mybir.dt.float32
f16 = mybir.dt.float16
f8 = mybir.dt.float8e4
Alu = mybir.AluOpType
Act = mybir.ActivationFunctionType
AX = mybir.AxisListType


def _build(rows: int, b_tune_val: float, stage: int = 99, reps: int = 1) -> bass.Bass:
    mch = min(MCH, rows)
    cpm = mch // 128                 # tiles per macro
    npair = cpm // 2                 # packed tile-pairs per macro
    nsub = mch // CHUNK              # sub-chunks per macro
    nmacro = rows // mch
    assert rows % mch == 0 and mch % CHUNK == 0 and cpm % 2 == 0
    nc = bacc.Bacc(None)

    hdt = f8 if H8 else f16
    # packed transposed xs: two host arrays [50, rows/2] (even/odd tiles)
    xse_ext = nc.declare_dram_parameter("xs_even", [NF, rows // 2], f16, isOutput=False)
    xso_ext = nc.declare_dram_parameter("xs_odd", [NF, rows // 2], f16, isOutput=False)
    h_ext = nc.declare_dram_parameter("hiddenT", [H, rows], hdt, isOutput=False)
    nv_ext = nc.declare_dram_parameter("naiveP", [128, rows // 128], f32, isOutput=False)
    tbl_ext = nc.declare_dram_parameter("btbl", [NF, 128], f16, isOutput=False)
    wtn_ext = nc.declare_dram_parameter("wtune", [H, 1], hdt, isOutput=False)
    out_ext = nc.declare_dram_parameter("out", [128, rows // 128], f32, isOutput=True)

    with tile.TileContext(nc) as tc:
        with (
            tc.tile_pool(name="consts", bufs=1) as cpool,
            tc.tile_pool(name="io", bufs=2) as io,
            tc.tile_pool(name="mid", bufs=3) as mid,
            tc.tile_pool(name="pst", bufs=2, space="PSUM") as pst,
            tc.tile_pool(name="psz", bufs=2, space="PSUM") as psz,
        ):
            btbl = cpool.tile([NF, 128], f16, tag="btbl")
            nc.sync.dma_start(btbl[:], tbl_ext[:])
            wtn = cpool.tile([H, 1], hdt, tag="wtn")
            nc.sync.dma_start(wtn[:], wtn_ext[:])
            btn = cpool.tile([128, 1], f32, tag="btn")
            nc.vector.memset(btn[:], b_tune_val)

            loop_cm = tc.For_i(0, reps, 1) if reps > 1 else contextlib.nullcontext()
            with loop_cm:
              for m in range(nmacro):
                m0 = m * mch
                t0 = m0 // 128              # first tile index of this macro
                # transposed xs for even/odd row-tiles, both at base partition 0
                xspe = io.tile([NF, npair * 128], f16, tag="xspe")
                nc.sync.dma_start(
                    xspe[:], xse_ext[:, (m0 // 2) : (m0 // 2) + npair * 128]
                )
                xspo = io.tile([NF, npair * 128], f16, tag="xspo")
                nc.sync.dma_start(
                    xspo[:], xso_ext[:, (m0 // 2) : (m0 // 2) + npair * 128]
                )
                hT = io.tile([H, mch], hdt, tag="hT")
                nc.scalar.dma_start(hT[:], h_ext[:, m0 : m0 + mch])
                nvm = io.tile([128, cpm], f32, tag="nvm")
                nc.sync.dma_start(nvm[:], nv_ext[:, t0 : t0 + cpm])
                resm = mid.tile([128, cpm], f32, tag="resm")

                if stage < 1:
                    nc.vector.tensor_copy(resm[:], nvm[:])
                    dum = mid.tile([128, 2], f32, tag="dum")
                    nc.vector.tensor_copy(dum[0:NF, 0:1], xspe[:, 0:1])
                    nc.vector.tensor_copy(dum[0:NF, 1:2], xspo[:, 0:1])
                    nc.vector.tensor_copy(dum[0:H, 1:2], hT[:, 0:1])
                else:
                  for sc in range(nsub):
                    c0 = sc * CPT
                    dt_ps = pst.tile([128, CPT, 128], f32, tag="dt_ps")
                    z_ps = psz.tile([128, CPT], f32, tag="z_ps")
                    for c in range(CPT):
                        tc_i = c0 + c            # tile index within macro
                        q, half = tc_i // 2, tc_i % 2
                        if EVENONLY:
                            half = 0
                        xst = xspe if half == 0 else xspo
                        nc.tensor.matmul(
                            dt_ps[:, c, :],
                            xst[:, q * 128 : (q + 1) * 128],
                            btbl[:],
                            start=True,
                            stop=True,
                        )
                        if not NOZ:
                            nc.tensor.matmul(
                                z_ps[:, c : c + 1],
                                hT[:, tc_i * 128 : (tc_i + 1) * 128],
                                wtn[:],
                                start=True,
                                stop=True,
                            )
                        else:
                            nc.vector.memset(z_ps[:, c : c + 1], 0.0)

                    if stage < 2:
                        nc.vector.tensor_copy(
                            resm[:, c0 : c0 + CPT], dt_ps[:, :, 0]
                        )
                        continue
                    # routing: row-min, one-hot, select t at argmin
                    rmin = mid.tile([128, CPT], f32, tag="rmin")
                    nc.vector.tensor_reduce(
                        rmin[:], dt_ps[:, :, 0:K], axis=AX.X, op=Alu.min
                    )
                    oh = mid.tile([128, CPT * K], mybir.dt.uint8 if SEL_MODE == "cpred" else f16, tag="oh")
                    oh3 = oh[:].rearrange("p (c f) -> p c f", c=CPT)
                    rb = rmin[:].unsqueeze(2).broadcast_to([128, CPT, K])
                    nc.vector.tensor_tensor(
                        oh3[:, :, :], dt_ps[:, :, 0:K], rb, op=Alu.is_equal
                    )
                    t16 = mid.tile([128, CPT * K], f16, tag="t16")
                    t163 = t16[:].rearrange("p (c f) -> p c f", c=CPT)
                    nc.scalar.copy(t163[:, :, :], dt_ps[:, :, K : 2 * K])
                    if stage < 3:
                        nc.vector.tensor_copy(resm[:, c0 : c0 + CPT], rmin[:])
                        nc.vector.tensor_add(
                            resm[:, c0 : c0 + 1], rmin[:, 0:1], oh3[:, 0, 0:1]
                        )
                        nc.vector.tensor_add(
                            resm[:, c0 : c0 + 1], rmin[:, 0:1], t163[:, 0, 0:1]
                        )
                        continue
                    xsn = mid.tile([128, CPT], f16 if SEL_MODE == "cpred" else f32, tag="xsn")
                    if SEL_MODE == "cpred":
                        xb = xsn[:].unsqueeze(2).broadcast_to([128, CPT, K])
                        nc.vector.copy_predicated(xb, oh3[:, :, :], t163[:, :, :])
                    else:
                        scr = mid.tile([128, CPT * K], f16, tag="scr")
                        sc3 = scr[:].rearrange("p (c f) -> p c f", c=CPT)
                        for c in range(CPT):
                            nc.vector.scalar_tensor_tensor(
                                out=sc3[:, c, :],
                                in0=oh3[:, c, :],
                                scalar=1.0,
                                in1=t163[:, c, :],
                                op0=Alu.bypass,
                                op1=Alu.mult,
                                accum_out=xsn[:, c : c + 1],
                            )

                    if stage < 4:
                        nc.vector.tensor_copy(resm[:, c0 : c0 + CPT], xsn[:])
                        continue
                    # tune head + blend
                    sig = mid.tile([128, CPT], f32, tag="sig")
                    nc.scalar.activation(
                        sig[:], z_ps[:, :], Act.Sigmoid, bias=btn[:, 0:1]
                    )
                    dd = mid.tile([128, CPT], f32, tag="dd")
                    nc.vector.tensor_sub(dd[:], nvm[:, c0 : c0 + CPT], xsn[:])
                    mm = mid.tile([128, CPT], f32, tag="mm")
                    nc.vector.tensor_mul(mm[:], dd[:], sig[:])
                    nc.vector.tensor_add(resm[:, c0 : c0 + CPT], xsn[:], mm[:])

                nc.scalar.dma_start(out_ext[:, t0 : t0 + cpm], resm[:])
    if not nc.is_finalized():
        nc.finalize()
    return nc


def _prep_tables(centers, W_tune, b_tune, W_sn, b_sn, running_sn_weight):
    centers = np.asarray(centers, np.float32)
    W_sn = np.asarray(W_sn, np.float32)
    b_sn = np.asarray(b_sn, np.float32)
    rsw = np.asarray(running_sn_weight, np.float32)
    c2 = (centers * centers).sum(1)
    WW = np.einsum("kn,nkd->kd", rsw, W_sn)
    BB = np.einsum("kn,nk->k", rsw, b_sn)
    tbl = np.zeros((NF, 128), np.float32)
    tbl[0:DX, 0:K] = -2.0 * centers.T
    tbl[DX, 0:K] = c2
    tbl[DX + 1 : DX + 1 + DS, K : 2 * K] = WW.T
    tbl[DX + 1 + DS, K : 2 * K] = BB
    wtn = np.asarray(W_tune, np.float32).reshape(H, 1)
    if H8:
        import ml_dtypes

        wtn = np.clip(wtn, -240, 240).astype(ml_dtypes.float8_e4m3fn)
    else:
        wtn = wtn.astype(np.float16)
    return tbl.astype(np.float16), wtn, float(np.asarray(b_tune).reshape(-1)[0])


def make_in_maps(inputs, rows_per_core, n_cores=NCORES):
    x = np.asarray(inputs["x"], np.float32)
    s = np.asarray(inputs["s"], np.float32)
    hidden = np.asarray(inputs["hidden"], np.float32)
    naive = np.asarray(inputs["naive_pred"], np.float32).reshape(-1)
    tbl, wtn, b_tune_val = _prep_tables(
        inputs["centers"], inputs["W_tune"], inputs["b_tune"],
        inputs["W_sn"], inputs["b_sn"], inputs["running_sn_weight"],
    )
    if H8:
        import ml_dtypes

        hcast = np.clip(hidden, -240, 240).astype(ml_dtypes.float8_e4m3fn)
    else:
        hcast = hidden.astype(np.float16)

    in_maps = []
    for i in range(n_cores):
        r0 = i * rows_per_core
        sl = slice(r0, r0 + rows_per_core)
        # xs1T [50, rows]: features x|1|s|1 on partitions
        xs1T = np.empty((NF, rows_per_core), np.float16)
        xs1T[0:DX] = x[sl].T
        xs1T[DX] = 1.0
        xs1T[DX + 1 : DX + 1 + DS] = s[sl].T
        xs1T[DX + 1 + DS] = 1.0
        # split into even/odd row-tiles: tile t = rows [128t, 128t+128)
        x4 = xs1T.reshape(NF, rows_per_core // 256, 2, 128)
        xs_even = np.ascontiguousarray(x4[:, :, 0, :]).reshape(NF, -1)
        xs_odd = np.ascontiguousarray(x4[:, :, 1, :]).reshape(NF, -1)
        hT = np.ascontiguousarray(hcast[sl].T)
        nvP = np.ascontiguousarray(
            naive[sl].reshape(rows_per_core // 128, 128).T
        )
        in_maps.append(
            {
                "xs_even": xs_even,
                "xs_odd": xs_odd,
                "hiddenT": hT,
                "naiveP": nvP,
                "btbl": tbl,
                "wtune": wtn,
            }
        )
    return in_maps, b_tune_val


def _unpack_out(bres, rows_per_core):
    outs = []
    for r in bres.results:
        o = r["out"]                       # [128, rows/128]: o[p, t] = row 128t+p
        outs.append(o.T.reshape(-1, 1))
    return np.concatenate(outs, axis=0)


def _run(inputs, rows_per_core=ROWS, n_cores=NCORES, trace=False, tmpdir=None):
    in_maps, b_tune_val = make_in_maps(inputs, rows_per_core, n_cores)
    nc = _build(rows_per_core, b_tune_val)
    bres = run_bass_kernel_spmd(
        nc, in_maps, core_ids=list(range(n_cores)), trace=trace, tmpdir=tmpdir
    )
    return _unpack_out(bres, rows_per_core), bres


def kernel(**inputs) -> np.ndarray:
    out, _ = _run(inputs)
    return out
